# revision 1
# baseline (speedup 1.0000x reference)
"""AnchorAttention distributed Bass kernel for 8 TRN2 NeuronCores.

Sharding: each core owns 512 of the 4096 output rows per batch:
  - 32 anchor rows per batch (4*32=128 anchor rows total per core)
  - 480 query rows per batch
Anchor inputs x[:, :K] are replicated (needed for K/V on every core).
No collectives: output shards are disjoint; host reassembles.

All device compute uses feature-on-partition ("transposed") layouts so no
on-device transposes are needed; the host pre-transposes inputs.
"""
import sys

for _p in ("/opt/trn_rl_repo", "/root/.axon_site/_ro/trn_rl_repo"):
    if _p not in sys.path:
        sys.path.insert(0, _p)

import numpy as np
import ml_dtypes

import concourse.bass as bass
import concourse.mybir as mybir
import concourse.tile as tile
from concourse import bacc
from concourse.bass_utils import run_bass_kernel_spmd

B, N, D = 4, 4096, 1024
H, HD = 16, 64
KA = 256                   # num anchor tokens
NCORES = 8
AQ = KA // NCORES          # 32 anchor rows per core per batch
QW = (N - KA) // NCORES    # 480 query rows per core per batch
R = AQ + QW                # 512 output rows per core per batch
SCALE = 1.0 / float(np.sqrt(HD))

F32 = mybir.dt.float32
BF16 = mybir.dt.bfloat16
EXP = mybir.ActivationFunctionType.Exp

BF = ml_dtypes.bfloat16


def build_graph(repeat=1, cfg=None):
    import os
    cfg = cfg or os.environ.get("KV_CFG", "E")
    nc = bacc.Bacc("TRN2", target_bir_lowering=False, debug=False,
                   num_devices=NCORES)

    # ---- external I/O (per-core shards) ----
    xaT_e = nc.dram_tensor("xaT", [B, D, KA], BF16, kind="ExternalInput")
    xqT_e = nc.dram_tensor("xqT", [B, D, QW], BF16, kind="ExternalInput")
    xamT_e = nc.dram_tensor("xamT", [D, B * AQ], BF16, kind="ExternalInput")
    wk_e = nc.dram_tensor("wk", [D, D], BF16, kind="ExternalInput")
    wv_e = nc.dram_tensor("wv", [D, D], BF16, kind="ExternalInput")
    wqt_e = nc.dram_tensor("wqt", [D, D], BF16, kind="ExternalInput")
    wq_e = nc.dram_tensor("wq", [D, D], BF16, kind="ExternalInput")
    wo_e = nc.dram_tensor("wo", [D, D], BF16, kind="ExternalInput")
    b3_e = nc.dram_tensor("b3_t", [128, 24], F32, kind="ExternalInput")
    b2_e = nc.dram_tensor("b2_r", [1, 2 * D], BF16, kind="ExternalInput")
    out_e = nc.dram_tensor("out", [B, R, D], F32, kind="ExternalOutput")

    def wload(pool, ext, name):
        t = pool.tile([128, 8, D], BF16, name=name)
        nc.sync.dma_start(t[:], ext.rearrange("(o p) e -> p o e", p=128))
        return t

    with tile.TileContext(nc) as tc:
      for _rep in range(repeat):
        with tc.tile_pool(name="perm", bufs=1) as perm, \
             tc.tile_pool(name="xa_stream", bufs=2) as pxa, \
             tc.tile_pool(name="xq_stream", bufs=8) as pxq, \
             tc.tile_pool(name="kv_pool", bufs=2) as pkv, \
             tc.tile_pool(name="q_pool", bufs=2) as pq, \
             tc.tile_pool(name="ctx_pool", bufs=(16 if cfg == "E" else 2)) as pctx, \
             tc.tile_pool(name="pool_p", bufs=6) as pool_p, \
             tc.tile_pool(name="pool_rec", bufs=6) as pool_rec, \
             tc.tile_pool(name="pool_rr", bufs=6) as pool_rr, \
             tc.tile_pool(name="pool_ot", bufs=3) as pool_ot, \
             tc.tile_pool(name="psum_proj", bufs=(2 if cfg == "A" else 1),
                          space="PSUM") as pp, \
             tc.tile_pool(name="ps_scores", bufs=(3 if cfg == "D" else 2),
                          space="PSUM") as psS, \
             tc.tile_pool(name="ps_ctx", bufs=2, space="PSUM") as psC, \
             tc.tile_pool(name="ps_sum", bufs=(1 if cfg == "D" else 2),
                          space="PSUM") as psM, \
             tc.tile_pool(name="ps_out", bufs=1, space="PSUM") as psO:

            # --- DMA priority order: smallest critical paths first ---
            wq_sb = wload(perm, wq_e, "wq_sb")
            xam_sb = perm.tile([128, 8, B * AQ], BF16)
            nc.sync.dma_start(xam_sb[:], xamT_e.rearrange("(o p) f -> p o f", p=128))
            b3_sb = perm.tile([128, 24], F32)
            nc.sync.dma_start(b3_sb[:], b3_e[:])
            b2_sb = perm.tile([1, 2 * D], BF16)
            nc.sync.dma_start(b2_sb[:], b2_e[:])
            bq_sb, bk_sb, bqt_sb = b3_sb[:, 0:8], b3_sb[:, 8:16], b3_sb[:, 16:24]
            ones_col = perm.tile([128, 1], BF16)
            nc.vector.memset(ones_col[:], 1.0)
            b2_bc = perm.tile([128, 2 * D], BF16)
            nc.gpsimd.partition_broadcast(b2_bc[:], b2_sb[:])
            bv_bc, bo_bc = b2_bc[:, 0:D], b2_bc[:, D:2 * D]

            wk_sb = wload(perm, wk_e, "wk_sb")
            xa_sb = {}

            def load_xa(b):
                if b >= B:
                    return
                t = pxa.tile([128, 8, KA], BF16, tag="xa", name=f"xa{b}")
                nc.sync.dma_start(
                    t[:], xaT_e[b].rearrange("(o p) f -> p o f", p=128))
                xa_sb[b] = t

            load_xa(0)
            wv_sb = wload(perm, wv_e, "wv_sb")
            wqt_sb = wload(perm, wqt_e, "wqt_sb")
            xq_chunks = {}

            def load_xq(b):
                if b >= B:
                    return
                cs = []
                for ch in range(4):
                    t = pxq.tile([128, 2, QW], BF16, tag="xq", name=f"xq{b}_{ch}")
                    nc.sync.dma_start(
                        t[:],
                        xqT_e[b].rearrange("(o p) f -> p o f", p=128)
                        [:, ch * 2:(ch + 1) * 2, :])
                    cs.append(t)
                xq_chunks[b] = cs

            load_xq(0)
            wo_sb = wload(perm, wo_e, "wo_sb")
            load_xa(1)
            load_xq(1)

            # --- anchor-q block (tiny, starts immediately) ---
            qaT_sb = perm.tile([128, 8, B * AQ], BF16)
            for et in range(8):
                psf = pp.tile([128, 512], F32, tag="proj", name="psqa")
                ps = psf[:, :B * AQ]
                for dt in range(8):
                    nc.tensor.matmul(
                        ps, wq_sb[:, dt, et * 128:(et + 1) * 128],
                        xam_sb[:, dt, :], start=(dt == 0), stop=(dt == 7))
                nc.vector.tensor_scalar_add(
                    qaT_sb[:, et, :], ps, bq_sb[:, et:et + 1])

            # --- fused per-batch pipeline ---
            for b in range(B):
                load_xa(b + 2)
                # kT_b[e, a]
                kT_b = pkv.tile([128, 8, KA], BF16, tag="kT", name=f"kT{b}")
                for et in range(8):
                    psf = pp.tile([128, 512], F32, tag="proj", name="psk")
                    ps = psf[:, :KA]
                    for dt in range(8):
                        nc.tensor.matmul(
                            ps, wk_sb[:, dt, et * 128:(et + 1) * 128],
                            xa_sb[b][:, dt, :], start=(dt == 0), stop=(dt == 7))
                    nc.scalar.add(kT_b[:, et, :], ps, bk_sb[:, et:et + 1])
                # v_b[a, (h, hd)]
                v_b = pkv.tile([128, 2, H, HD], BF16, tag="v", name=f"v{b}")
                for at in range(2):
                    for en in range(2):
                        ps = pp.tile([128, 512], F32, tag="proj", name="psv")
                        for dt in range(8):
                            nc.tensor.matmul(
                                ps, xa_sb[b][:, dt, at * 128:(at + 1) * 128],
                                wv_sb[:, dt, en * 512:(en + 1) * 512],
                                start=(dt == 0), stop=(dt == 7))
                        nc.vector.tensor_add(
                            v_b[:, at, en * 8:(en + 1) * 8, :],
                            ps.rearrange("p (h x) -> p h x", x=HD),
                            bv_bc[:, en * 512:(en + 1) * 512].rearrange(
                                "p (h x) -> p h x", x=HD))
                # qT_b[e, r]
                load_xq(b + 1)
                qT_b = pq.tile([128, 8, R], BF16, tag="qT", name=f"qT{b}")
                chunks = xq_chunks[b]
                for et in range(8):
                    psf = pp.tile([128, 512], F32, tag="proj", name="psq")
                    ps = psf[:, :QW]
                    for dt in range(8):
                        nc.tensor.matmul(
                            ps, wqt_sb[:, dt, et * 128:(et + 1) * 128],
                            chunks[dt // 2][:, dt % 2, :],
                            start=(dt == 0), stop=(dt == 7))
                    nc.scalar.add(qT_b[:, et, AQ:R], ps, bqt_sb[:, et:et + 1])
                    nc.vector.tensor_copy(
                        qT_b[:, et, 0:AQ], qaT_sb[:, et, b * AQ:(b + 1) * AQ])

                # attention: head pairs per e-tile, normalization tails
                # flushed per 2-et group so DVE recips aren't head-of-line
                # blocked behind muls waiting on the gpsimd broadcast.
                # ctxT split per e-tile so the output projection can start
                # accumulating as soon as early e-tiles are normalized.
                if cfg == "E":
                    ctx_ts = [pctx.tile([128, R], BF16, tag="ctxT",
                                        name=f"ctxT{b}_{i}") for i in range(8)]
                else:
                    ctxT_b = pctx.tile([128, 8, R], BF16, tag="ctxTm",
                                       name=f"ctxT{b}")
                    ctx_ts = [ctxT_b[:, i, :] for i in range(8)]
                pend = []
                for et in range(8):
                    ps_c = psC.tile([128, 512], F32, tag="c")
                    p_t = {}

                    def scores(par):
                        po = par * 64
                        kT_h = kT_b[po:po + 64, et, :]
                        qT_h = qT_b[po:po + 64, et, :]
                        for at in range(2):
                            ps_s = psS.tile([128, 512], F32, tag="s")
                            nc.tensor.matmul(
                                ps_s, kT_h[:, at * 128:(at + 1) * 128], qT_h,
                                start=True, stop=True, tile_position=(po, 0))
                            pt = pool_p.tile([128, 512], BF16, tag="p")
                            nc.scalar.activation(pt[:], ps_s, EXP, scale=SCALE)
                            p_t[par, at] = pt

                    def tail(par):
                        h = 2 * et + par
                        po = par * 64
                        for at in range(2):
                            nc.tensor.matmul(
                                ps_c[po:po + 64, :], v_b[:, at, h, :],
                                p_t[par, at][:],
                                start=(at == 0), stop=(at == 1),
                                tile_position=(0, po))
                        ps_m = psM.tile([1, 512], F32, tag="m")
                        for at in range(2):
                            nc.tensor.matmul(
                                ps_m[:], ones_col[:, :], p_t[par, at][:],
                                start=(at == 0), stop=(at == 1))
                        rr = pool_rr.tile([1, 512], F32, tag="rr")
                        nc.vector.reciprocal(rr[:], ps_m[:])
                        rec = pool_rec.tile([128, 512], F32, tag="rec")
                        # even heads only need partitions 0..64 of the
                        # broadcast (base-0 output is the supported form)
                        nc.gpsimd.partition_broadcast(
                            rec[0:(po + 64 if cfg == "E" else 128), :], rr[:])
                        pend.append((et, po, ps_c, rec))

                    if cfg == "D":
                        scores(0); scores(1); tail(0); tail(1)
                    else:
                        scores(0); tail(0); scores(1); tail(1)
                    if et % 2 == 1:
                        for pet, ppo, pps_c, prec in pend:
                            nc.vector.tensor_mul(
                                ctx_ts[pet][ppo:ppo + 64, :],
                                pps_c[ppo:ppo + 64, :], prec[ppo:ppo + 64, :])
                        pend = []

                # output projection
                for rt in range(4):
                    ot = pool_ot.tile([128, D], F32, tag="ot")
                    for eo in range(2):
                        ps_o = psO.tile([128, 512], F32, tag="o")
                        for ct in range(8):
                            nc.tensor.matmul(
                                ps_o, ctx_ts[ct][:, rt * 128:(rt + 1) * 128],
                                wo_sb[:, ct, eo * 512:(eo + 1) * 512],
                                start=(ct == 0), stop=(ct == 7))
                        nc.vector.tensor_add(
                            ot[:, eo * 512:(eo + 1) * 512], ps_o,
                            bo_bc[:, eo * 512:(eo + 1) * 512])
                    nc.sync.dma_start(
                        out_e[b, rt * 128:(rt + 1) * 128, :], ot[:])

    nc.compile()
    return nc


def host_prep(x, Wq, bq, Wk, bk, Wv, bv, Wqt, bqt, Wo, bo):
    """Build per-core in_maps from full inputs."""
    x = np.asarray(x, dtype=np.float32)
    bf = lambda a: np.ascontiguousarray(np.asarray(a, np.float32)).astype(BF)
    xa = x[:, :KA, :]                                      # [B, KA, D]
    xaT = np.ascontiguousarray(xa.transpose(0, 2, 1))      # [B, D, KA]
    bias_t = lambda v: np.asarray(v, np.float32).reshape(8, 128).T  # [128, 8]
    b3 = np.ascontiguousarray(
        np.concatenate([bias_t(bq), bias_t(bk), bias_t(bqt)], axis=1))
    b2 = np.concatenate([np.asarray(bv, np.float32),
                         np.asarray(bo, np.float32)]).reshape(1, 2 * D)
    common = {
        "xaT": xaT.astype(BF),
        "wk": bf(Wk), "wv": bf(Wv), "wqt": bf(Wqt),
        "wq": bf(Wq), "wo": bf(Wo),
        "b3_t": b3, "b2_r": b2.astype(BF),
    }
    in_maps = []
    for c in range(NCORES):
        xq = x[:, KA + c * QW:KA + (c + 1) * QW, :]        # [B, QW, D]
        xqT = np.ascontiguousarray(xq.transpose(0, 2, 1)).astype(BF)
        xam = np.concatenate(
            [xaT[b][:, c * AQ:(c + 1) * AQ] for b in range(B)], axis=1)
        in_maps.append(dict(common, xqT=xqT,
                            xamT=np.ascontiguousarray(xam).astype(BF)))
    return in_maps


def assemble(results):
    """[core][b, r, e] shards -> full [B, N, D]."""
    out = np.empty((B, N, D), dtype=np.float32)
    for c in range(NCORES):
        o = results[c]["out"]
        for b in range(B):
            out[b, c * AQ:(c + 1) * AQ] = o[b, :AQ]
            out[b, KA + c * QW:KA + (c + 1) * QW] = o[b, AQ:]
    return out


def kernel(x, Wq, bq, Wk, bk, Wv, bv, Wqt, bqt, Wo, bo, num_anchor_tokens):
    assert int(num_anchor_tokens) == KA, f"expected {KA} anchors"
    in_maps = host_prep(x, Wq, bq, Wk, bk, Wv, bv, Wqt, bqt, Wo, bo)
    nc = build_graph()
    res = run_bass_kernel_spmd(nc, in_maps, core_ids=list(range(NCORES)))
    return assemble(res.results)



# revision 6
# speedup vs baseline: 1.0168x; 1.0168x over previous
"""AnchorAttention distributed Bass kernel for 8 TRN2 NeuronCores.

Sharding: each core owns 512 of the 4096 output rows per batch:
  - 32 anchor rows per batch (4*32=128 anchor rows total per core)
  - 480 query rows per batch
Anchor inputs x[:, :K] are replicated (needed for K/V on every core).
No collectives: output shards are disjoint; host reassembles.

All device compute uses feature-on-partition ("transposed") layouts so no
on-device transposes are needed; the host pre-transposes inputs.
"""
import sys

for _p in ("/opt/trn_rl_repo", "/root/.axon_site/_ro/trn_rl_repo"):
    if _p not in sys.path:
        sys.path.insert(0, _p)

import numpy as np
import ml_dtypes

import concourse.bass as bass
import concourse.mybir as mybir
import concourse.tile as tile
from concourse import bacc
from concourse.bass_utils import run_bass_kernel_spmd

B, N, D = 4, 4096, 1024
H, HD = 16, 64
KA = 256                   # num anchor tokens
NCORES = 8
AQ = KA // NCORES          # 32 anchor rows per core per batch
QW = (N - KA) // NCORES    # 480 query rows per core per batch
R = AQ + QW                # 512 output rows per core per batch
SCALE = 1.0 / float(np.sqrt(HD))

F32 = mybir.dt.float32
BF16 = mybir.dt.bfloat16
EXP = mybir.ActivationFunctionType.Exp

BF = ml_dtypes.bfloat16


def build_graph(repeat=1, cfg=None):
    import os
    cfg = cfg or os.environ.get("KV_CFG", "B")
    nc = bacc.Bacc("TRN2", target_bir_lowering=False, debug=False,
                   num_devices=NCORES)

    # ---- external I/O (per-core shards) ----
    xaT_e = nc.dram_tensor("xaT", [B, D, KA], BF16, kind="ExternalInput")
    xqT_e = nc.dram_tensor("xqT", [B, D, QW], BF16, kind="ExternalInput")
    xamT_e = nc.dram_tensor("xamT", [D, B * AQ], BF16, kind="ExternalInput")
    wk_e = nc.dram_tensor("wk", [D, D], BF16, kind="ExternalInput")
    wv_e = nc.dram_tensor("wv", [D, D], BF16, kind="ExternalInput")
    wqt_e = nc.dram_tensor("wqt", [D, D], BF16, kind="ExternalInput")
    wq_e = nc.dram_tensor("wq", [D, D], BF16, kind="ExternalInput")
    wo_e = nc.dram_tensor("wo", [D, D], BF16, kind="ExternalInput")
    b3_e = nc.dram_tensor("b3_t", [128, 24], F32, kind="ExternalInput")
    b2_e = nc.dram_tensor("b2_r", [1, 2 * D], BF16, kind="ExternalInput")
    out_e = nc.dram_tensor("out", [B, R, D], F32, kind="ExternalOutput")

    def wload(pool, ext, name):
        t = pool.tile([128, 8, D], BF16, name=name)
        nc.sync.dma_start(t[:], ext.rearrange("(o p) e -> p o e", p=128))
        return t

    with tile.TileContext(nc) as tc:
      for _rep in range(repeat):
        with tc.tile_pool(name="perm", bufs=1) as perm, \
             tc.tile_pool(name="xa_stream", bufs=2) as pxa, \
             tc.tile_pool(name="xq_stream", bufs=8) as pxq, \
             tc.tile_pool(name="kv_pool", bufs=2) as pkv, \
             tc.tile_pool(name="q_pool", bufs=2) as pq, \
             tc.tile_pool(name="ctx_pool", bufs=16) as pctx, \
             tc.tile_pool(name="pool_p", bufs=6) as pool_p, \
             tc.tile_pool(name="pool_rec", bufs=6) as pool_rec, \
             tc.tile_pool(name="pool_rr", bufs=6) as pool_rr, \
             tc.tile_pool(name="pool_ot", bufs=3) as pool_ot, \
             tc.tile_pool(name="psum_proj", bufs=2, space="PSUM") as pp, \
             tc.tile_pool(name="ps_scores", bufs=2, space="PSUM") as psS, \
             tc.tile_pool(name="ps_ctx", bufs=3, space="PSUM") as psC, \
             tc.tile_pool(name="ps_out", bufs=1, space="PSUM") as psO:

            # --- DMA priority order: smallest critical paths first ---
            wq_sb = wload(perm, wq_e, "wq_sb")
            xam_sb = perm.tile([128, 8, B * AQ], BF16)
            nc.sync.dma_start(xam_sb[:], xamT_e.rearrange("(o p) f -> p o f", p=128))
            b3_sb = perm.tile([128, 24], F32)
            nc.sync.dma_start(b3_sb[:], b3_e[:])
            b2_sb = perm.tile([1, 2 * D], BF16)
            nc.sync.dma_start(b2_sb[:], b2_e[:])
            bq_sb, bk_sb, bqt_sb = b3_sb[:, 0:8], b3_sb[:, 8:16], b3_sb[:, 16:24]
            b2_bc = perm.tile([128, 2 * D], BF16)
            nc.gpsimd.partition_broadcast(b2_bc[:], b2_sb[:])
            bv_bc, bo_bc = b2_bc[:, 0:D], b2_bc[:, D:2 * D]

            wk_sb = wload(perm, wk_e, "wk_sb")
            xa_sb = {}

            def load_xa(b):
                if b >= B:
                    return
                t = pxa.tile([128, 8, KA], BF16, tag="xa", name=f"xa{b}")
                nc.sync.dma_start(
                    t[:], xaT_e[b].rearrange("(o p) f -> p o f", p=128))
                xa_sb[b] = t

            load_xa(0)
            wv_sb = wload(perm, wv_e, "wv_sb")
            wqt_sb = wload(perm, wqt_e, "wqt_sb")
            xq_chunks = {}

            def load_xq(b):
                if b >= B:
                    return
                cs = []
                for ch in range(4):
                    t = pxq.tile([128, 2, QW], BF16, tag="xq", name=f"xq{b}_{ch}")
                    nc.sync.dma_start(
                        t[:],
                        xqT_e[b].rearrange("(o p) f -> p o f", p=128)
                        [:, ch * 2:(ch + 1) * 2, :])
                    cs.append(t)
                xq_chunks[b] = cs

            load_xq(0)
            wo_sb = wload(perm, wo_e, "wo_sb")
            load_xa(1)
            load_xq(1)

            # --- anchor-q block (tiny, starts immediately) ---
            qaT_sb = perm.tile([128, 8, B * AQ], BF16)
            for et in range(8):
                psf = pp.tile([128, 512], F32, tag="proj", name="psqa")
                ps = psf[:, :B * AQ]
                for dt in range(8):
                    nc.tensor.matmul(
                        ps, wq_sb[:, dt, et * 128:(et + 1) * 128],
                        xam_sb[:, dt, :], start=(dt == 0), stop=(dt == 7))
                nc.vector.tensor_scalar_add(
                    qaT_sb[:, et, :], ps, bq_sb[:, et:et + 1])

            # --- fused per-batch pipeline ---
            for b in range(B):
                load_xa(b + 2)
                # kT_b[e, a]
                kT_b = pkv.tile([128, 8, KA], BF16, tag="kT", name=f"kT{b}")
                for et in range(8):
                    psf = pp.tile([128, 512], F32, tag="proj", name="psk")
                    ps = psf[:, :KA]
                    for dt in range(8):
                        nc.tensor.matmul(
                            ps, wk_sb[:, dt, et * 128:(et + 1) * 128],
                            xa_sb[b][:, dt, :], start=(dt == 0), stop=(dt == 7))
                    nc.scalar.add(kT_b[:, et, :], ps, bk_sb[:, et:et + 1])
                # v_b[a, (h, hd+1)] — last column is ones so the ctx matmul
                # also produces the softmax denominator in psum row HD
                v_b = pkv.tile([128, 2, H, HD + 1], BF16, tag="v", name=f"v{b}")
                nc.vector.memset(v_b[:, :, :, HD:HD + 1], 1.0)
                for at in range(2):
                    for en in range(2):
                        ps = pp.tile([128, 512], F32, tag="proj", name="psv")
                        for dt in range(8):
                            nc.tensor.matmul(
                                ps, xa_sb[b][:, dt, at * 128:(at + 1) * 128],
                                wv_sb[:, dt, en * 512:(en + 1) * 512],
                                start=(dt == 0), stop=(dt == 7))
                        nc.vector.tensor_add(
                            v_b[:, at, en * 8:(en + 1) * 8, :HD],
                            ps.rearrange("p (h x) -> p h x", x=HD),
                            bv_bc[:, en * 512:(en + 1) * 512].rearrange(
                                "p (h x) -> p h x", x=HD))
                # qT_b[e, r]
                load_xq(b + 1)
                qT_b = pq.tile([128, 8, R], BF16, tag="qT", name=f"qT{b}")
                chunks = xq_chunks[b]
                for et in range(8):
                    psf = pp.tile([128, 512], F32, tag="proj", name="psq")
                    ps = psf[:, :QW]
                    for dt in range(8):
                        nc.tensor.matmul(
                            ps, wqt_sb[:, dt, et * 128:(et + 1) * 128],
                            chunks[dt // 2][:, dt % 2, :],
                            start=(dt == 0), stop=(dt == 7))
                    nc.scalar.add(qT_b[:, et, AQ:R], ps, bqt_sb[:, et:et + 1])
                    nc.vector.tensor_copy(
                        qT_b[:, et, 0:AQ], qaT_sb[:, et, b * AQ:(b + 1) * AQ])

                # attention: per-head psum banks; the V ones-column makes the
                # ctx matmul emit the softmax denominator in row HD, so no
                # separate PE sum matmuls are needed.
                ctx_ts = [pctx.tile([128, R], BF16, tag="ctxT",
                                    name=f"ctxT{b}_{i}") for i in range(8)]
                for et in range(8):
                    p_t = {}

                    def scores(par):
                        po = par * 64
                        kT_h = kT_b[po:po + 64, et, :]
                        qT_h = qT_b[po:po + 64, et, :]
                        for at in range(2):
                            ps_s = psS.tile([128, 512], F32, tag="s")
                            nc.tensor.matmul(
                                ps_s, kT_h[:, at * 128:(at + 1) * 128], qT_h,
                                start=True, stop=True, tile_position=(po, 0))
                            pt = pool_p.tile([128, 512], BF16, tag="p")
                            nc.scalar.activation(pt[:], ps_s, EXP, scale=SCALE)
                            p_t[par, at] = pt

                    def tail(par):
                        h = 2 * et + par
                        po = par * 64
                        ps_c = psC.tile([128, 512], F32, tag="c")
                        for at in range(2):
                            nc.tensor.matmul(
                                ps_c[0:HD + 1, :], v_b[:, at, h, :],
                                p_t[par, at][:],
                                start=(at == 0), stop=(at == 1))
                        rr = pool_rr.tile([1, 512], F32, tag="rr")
                        nc.vector.reciprocal(rr[:], ps_c[HD:HD + 1, :])
                        rec = pool_rec.tile([64, 512], F32, tag="rec")
                        nc.gpsimd.partition_broadcast(rec[:], rr[:])
                        nc.vector.tensor_mul(
                            ctx_ts[et][po:po + 64, :],
                            ps_c[0:64, :], rec[:, :])

                    scores(0); tail(0); scores(1); tail(1)

                # output projection
                for rt in range(4):
                    ot = pool_ot.tile([128, D], F32, tag="ot")
                    for eo in range(2):
                        ps_o = psO.tile([128, 512], F32, tag="o")
                        for ct in range(8):
                            nc.tensor.matmul(
                                ps_o, ctx_ts[ct][:, rt * 128:(rt + 1) * 128],
                                wo_sb[:, ct, eo * 512:(eo + 1) * 512],
                                start=(ct == 0), stop=(ct == 7))
                        nc.vector.tensor_add(
                            ot[:, eo * 512:(eo + 1) * 512], ps_o,
                            bo_bc[:, eo * 512:(eo + 1) * 512])
                    nc.sync.dma_start(
                        out_e[b, rt * 128:(rt + 1) * 128, :], ot[:])

    nc.compile()
    return nc


def host_prep(x, Wq, bq, Wk, bk, Wv, bv, Wqt, bqt, Wo, bo):
    """Build per-core in_maps from full inputs."""
    x = np.asarray(x, dtype=np.float32)
    bf = lambda a: np.ascontiguousarray(np.asarray(a, np.float32)).astype(BF)
    xa = x[:, :KA, :]                                      # [B, KA, D]
    xaT = np.ascontiguousarray(xa.transpose(0, 2, 1))      # [B, D, KA]
    bias_t = lambda v: np.asarray(v, np.float32).reshape(8, 128).T  # [128, 8]
    b3 = np.ascontiguousarray(
        np.concatenate([bias_t(bq), bias_t(bk), bias_t(bqt)], axis=1))
    b2 = np.concatenate([np.asarray(bv, np.float32),
                         np.asarray(bo, np.float32)]).reshape(1, 2 * D)
    common = {
        "xaT": xaT.astype(BF),
        "wk": bf(Wk), "wv": bf(Wv), "wqt": bf(Wqt),
        "wq": bf(Wq), "wo": bf(Wo),
        "b3_t": b3, "b2_r": b2.astype(BF),
    }
    in_maps = []
    for c in range(NCORES):
        xq = x[:, KA + c * QW:KA + (c + 1) * QW, :]        # [B, QW, D]
        xqT = np.ascontiguousarray(xq.transpose(0, 2, 1)).astype(BF)
        xam = np.concatenate(
            [xaT[b][:, c * AQ:(c + 1) * AQ] for b in range(B)], axis=1)
        in_maps.append(dict(common, xqT=xqT,
                            xamT=np.ascontiguousarray(xam).astype(BF)))
    return in_maps


def assemble(results):
    """[core][b, r, e] shards -> full [B, N, D]."""
    out = np.empty((B, N, D), dtype=np.float32)
    for c in range(NCORES):
        o = results[c]["out"]
        for b in range(B):
            out[b, c * AQ:(c + 1) * AQ] = o[b, :AQ]
            out[b, KA + c * QW:KA + (c + 1) * QW] = o[b, AQ:]
    return out


def kernel(x, Wq, bq, Wk, bk, Wv, bv, Wqt, bqt, Wo, bo, num_anchor_tokens):
    assert int(num_anchor_tokens) == KA, f"expected {KA} anchors"
    in_maps = host_prep(x, Wq, bq, Wk, bk, Wv, bv, Wqt, bqt, Wo, bo)
    nc = build_graph()
    res = run_bass_kernel_spmd(nc, in_maps, core_ids=list(range(NCORES)))
    return assemble(res.results)

